# revision 1
# baseline (speedup 1.0000x reference)
"""Trainium2 Bass kernel for nn_DUSPSA (SPSA on f(x)=x0^2+Q*x1^2, 1000 iters).

The per-step SPSA update is linear in x given the Rademacher signs:
    x' = M_k(p) x,  M_k = [[c1_k, -c2_k p],[-c3_k p, c4_k]],  p = d0*d1 in {+-1}
(exact algebra of the reference's finite-difference step).  Per 128-step block
we build per-element 2x2 transfer matrices by parallel doubling (all large
free-dim DVE ops), then apply the block matrix to the running state.
Data-parallel over the batch across 8 cores; all heavy tensor work on device.

Note: consecutive dependent DVE ops in raw bass exhibit a read-after-write
pipeline hazard; every dependent pair below is separated by >=2 ops.
"""
import numpy as np

import concourse.bass as bass
import concourse.mybir as mybir
from concourse.bass_utils import run_bass_kernel_spmd

ALPHA, GAMMA, Q = 0.602, 0.101, 8.0
N_CORES = 8
BS = 16384
BPC = BS // N_CORES          # 2048 batch elements per core
P = 128                      # partitions
C = BPC // P                 # 16 batch columns per partition
NIT = 1000
NPAD = 1024
T = 128                      # steps per block
NB = NPAD // T               # 8 blocks
NPAIR = T // 2               # 64 level-1 pairs per block
NLEV = 7                     # 64 -> 32 -> 16 -> 8 -> 4 -> 2 -> 1
SIZES = [NPAIR >> (l - 1) for l in range(1, NLEV + 1)]
f32 = mybir.dt.float32
i32 = mybir.dt.int32
MUL = mybir.AluOpType.mult
ADD = mybir.AluOpType.add
XOR = mybir.AluOpType.logical_xor

_CACHED = {}

import os
FP16 = os.environ.get("DUSPSA_FP16", "0") == "1"
f16 = mybir.dt.float16


def _build_nc():
    import contextlib

    nc = bass.Bass("TRN2", target_bir_lowering=False, debug=False)
    delta = nc.declare_dram_parameter("delta", [P, NB * T * C * 2], i32, isOutput=False)
    xin = nc.declare_dram_parameter("xin", [P, 2 * C], f32, isOutput=False)
    consts = nc.declare_dram_parameter("consts", [1, NB * 8 * NPAIR], f16 if FP16 else f32, isOutput=False)
    yout = nc.declare_dram_parameter("yout", [P, 2 * C], f32, isOutput=True)

    stack = contextlib.ExitStack()
    with stack:
        sb = lambda name, shape, dt=f32: stack.enter_context(nc.sbuf_tensor(name, shape, dt))
        gdt = f16 if FP16 else f32
        ti0 = sb("ti0", [P, T * C * 2], i32)
        ti1 = sb("ti1", [P, T * C * 2], i32)
        xr = sb("xr", [P, T * C], i32)
        pp0 = sb("pp0", [P, T * C], gdt)
        pp1 = sb("pp1", [P, T * C], gdt)
        cst = sb("cst", [P, NB * 8 * NPAIR], gdt)
        xt = sb("xt", [P, 2 * C])
        out_stage = sb("out_stage", [P, 2 * C])
        y0, y1, y0b, y1b = (sb(n, [P, C]) for n in ("y0", "y1", "y0b", "y1b"))
        a1, a2, a3, a4 = (sb(n, [P, C]) for n in ("a1", "a2", "a3", "a4"))
        dummy = sb("spacer_t", [P, C])
        g_tiles = [
            [sb(f"g{l}_{e}", [P, s * C], f32 if l == NLEV - 1 else gdt) for e in range(4)]
            for l, s in enumerate(SIZES)
        ]
        tmp_tiles = [sb(f"tmp{i}", [P, NPAIR * C], gdt) for i in range(8)]
        tmpf_tiles = [sb(f"tmpf{i}", [P, C], f32) for i in range(8)]
        dma_sem = stack.enter_context(nc.semaphore("dma"))
        done_sem = stack.enter_context(nc.semaphore("done"))
        gp_p = stack.enter_context(nc.semaphore("gp_p"))
        gp_xor = stack.enter_context(nc.semaphore("gp_xor"))
        dve_l1 = stack.enter_context(nc.semaphore("dve_l1"))
        block = stack.enter_context(nc.Block())

        tis = [ti0, ti1]

        def cst_bc(b, idx, n=NPAIR):
            base = (b * 8 + idx) * NPAIR
            return cst[:, base : base + n].unsqueeze(2).broadcast_to((P, n, C))

        @block.sync
        def _(sync):
            sync.dma_start(out=xt[:], in_=xin[:]).then_inc(dma_sem, 16)
            sync.dma_start(
                out=cst[:], in_=consts[0:1, :].partition_broadcast(P).squeeze(1)
            ).then_inc(dma_sem, 16)
            for b in range(NB):
                if b >= 2:
                    sync.wait_ge(done_sem, b - 1)  # buffer b%2 freed by xor(b-2)
                sync.dma_start(
                    out=tis[b % 2][:], in_=delta[:, b * T * C * 2 : (b + 1) * T * C * 2]
                ).then_inc(dma_sem, 16)
            sync.wait_ge(done_sem, NB + 1)
            sync.dma_start(out=yout[:], in_=out_stage[:]).then_inc(dma_sem, 16)

        @block.vector
        def _(vector):
            def p3(ap, nk):
                return ap.rearrange("p (k c) -> p k c", c=C)

            pps = [pp0, pp1]

            def emit_xor(b):
                vector.wait_ge(dma_sem, 32 + 16 * (b + 1))
                ti = tis[b % 2]
                vector.tensor_tensor(
                    xr[:], ti[:, 0 : 2 * T * C : 2], ti[:, 1 : 2 * T * C : 2], XOR
                ).then_inc(done_sem, 1)

            def emit_p(b):
                vector.tensor_scalar(pps[b % 2][:], xr[:], -2.0, 1.0, MUL, ADD)

            # prologue: block 0's p with hazard spacing
            vector.wait_ge(dma_sem, 32)
            emit_xor(0)
            vector.tensor_scalar(y0[:], xt[:, 0 : 2 * C : 2], 20.0, -10.0, MUL, ADD)
            vector.tensor_scalar(y1[:], xt[:, 1 : 2 * C : 2], 20.0, -10.0, MUL, ADD)
            emit_p(0)
            vector.tensor_copy(dummy[:], xt[:, 0:C])
            vector.tensor_copy(a1[:], xt[:, C : 2 * C])

            ys = [(y0, y1), (y0b, y1b)]

            for b in range(NB):
                # ---- L1: pair matrices from p ----
                pp = pps[b % 2]
                pE = p3(pp[:], T)[:, 0 : T : 2, :]
                pO = p3(pp[:], T)[:, 1 : T : 2, :]
                r, u, v, u2, v2, w, w2, sp = tmp_tiles
                G = g_tiles[0]
                vector.tensor_tensor(p3(r[:], NPAIR), pE, pO, MUL)
                vector.tensor_tensor(p3(u[:], NPAIR), pE, cst_bc(b, 0), MUL)   # g1*pE
                vector.tensor_tensor(p3(v[:], NPAIR), pO, cst_bc(b, 1), MUL)   # g2*pO
                vector.tensor_tensor(p3(u2[:], NPAIR), pE, cst_bc(b, 2), MUL)  # h1*pE
                vector.tensor_tensor(p3(v2[:], NPAIR), pO, cst_bc(b, 3), MUL)  # h2*pO
                vector.tensor_tensor(p3(w[:], NPAIR), p3(r[:], NPAIR), cst_bc(b, 4), MUL)
                vector.tensor_tensor(p3(w2[:], NPAIR), p3(r[:], NPAIR), cst_bc(b, 6), MUL)
                vector.tensor_tensor(G[1][:], u[:], v[:], ADD)                 # G01
                vector.tensor_tensor(G[2][:], u2[:], v2[:], ADD)               # G10
                vector.tensor_tensor(p3(G[0][:], NPAIR), p3(w[:], NPAIR), cst_bc(b, 5), ADD)
                vector.tensor_tensor(p3(G[3][:], NPAIR), p3(w2[:], NPAIR), cst_bc(b, 7), ADD)

                if b + 1 < NB:
                    emit_xor(b + 1)

                # ---- doubling levels ----
                for l in range(1, NLEV):
                    m = SIZES[l]
                    Gp, Gn = g_tiles[l - 1], g_tiles[l]
                    E = [p3(Gp[e][:], 2 * m)[:, 0 : 2 * m : 2, :] for e in range(4)]
                    F = [p3(Gp[e][:], 2 * m)[:, 1 : 2 * m : 2, :] for e in range(4)]
                    tsrc = tmpf_tiles if l == NLEV - 1 else tmp_tiles
                    t1, t2, t3, t4, t5, t6, t7, t8 = [
                        p3(t[:, 0 : m * C], m) for t in tsrc
                    ]
                    O = [p3(Gn[e][:], m) for e in range(4)]
                    vector.tensor_tensor(t2, F[1], E[2], MUL)   # F01*E10
                    vector.tensor_tensor(t7, F[2], E[1], MUL)   # F10*E01
                    if l == 1 and b + 1 < NB:
                        emit_p(b + 1)
                    vector.tensor_tensor(t1, F[0], E[0], MUL)   # F00*E00
                    vector.tensor_tensor(t5, F[2], E[0], MUL)   # F10*E00
                    vector.tensor_tensor(t3, F[0], E[1], MUL)   # F00*E01
                    vector.tensor_tensor(t8, F[3], E[3], MUL)   # F11*E11
                    vector.tensor_tensor(t4, F[1], E[3], MUL)   # F01*E11
                    vector.tensor_tensor(t6, F[3], E[2], MUL)   # F11*E10
                    vector.tensor_tensor(O[0], t1, t2, ADD)
                    vector.tensor_tensor(O[1], t3, t4, ADD)
                    vector.tensor_tensor(O[2], t5, t6, ADD)
                    vector.tensor_tensor(O[3], t7, t8, ADD)

                # ---- apply block matrix to state ----
                yc0, yc1 = ys[b % 2]
                yn0, yn1 = ys[(b + 1) % 2]
                GL = g_tiles[NLEV - 1]
                vector.tensor_tensor(a1[:], GL[0][:], yc0[:], MUL)
                vector.tensor_tensor(a2[:], GL[1][:], yc1[:], MUL)
                vector.tensor_tensor(a3[:], GL[2][:], yc0[:], MUL)
                vector.tensor_tensor(a4[:], GL[3][:], yc1[:], MUL)
                vector.tensor_copy(dummy[:], a1[:])  # hazard spacer
                vector.tensor_tensor(yn0[:], a1[:], a2[:], ADD)
                vector.tensor_tensor(yn1[:], a3[:], a4[:], ADD)

            yf0, yf1 = ys[NB % 2]
            vector.tensor_copy(dummy[:], yf0[:])  # hazard spacer
            vector.tensor_copy(out_stage[:, 0:C], yf0[:])
            vector.tensor_copy(out_stage[:, C : 2 * C], yf1[:]).then_inc(done_sem, 1)

    return nc


def _host_constants(a, c, num_itr):
    n = int(num_itr)
    A = int(np.floor(0.1 * n))
    k = np.arange(1, n + 1, dtype=np.float64)
    ak = a.astype(np.float64) / (k + 1.0 + A) ** ALPHA
    c1 = 1.0 - 2.0 * ak
    c4 = 1.0 - 2.0 * ak * Q
    c2 = 2.0 * ak * Q
    c3 = 2.0 * ak
    pad = NPAD - n
    c1 = np.concatenate([c1, np.ones(pad)]).astype(np.float32)
    c4 = np.concatenate([c4, np.ones(pad)]).astype(np.float32)
    c2 = np.concatenate([c2, np.zeros(pad)]).astype(np.float32)
    c3 = np.concatenate([c3, np.zeros(pad)]).astype(np.float32)
    e = np.arange(0, NPAD, 2)
    o = e + 1
    # G = M_o @ M_e, M = [[c1, -c2 p],[-c3 p, c4]]
    g1 = -(c1[o] * c2[e])      # * pE  -> G01
    g2 = -(c2[o] * c4[e])      # * pO
    h1 = -(c4[o] * c3[e])      # * pE  -> G10
    h2 = -(c3[o] * c1[e])      # * pO
    beta = c2[o] * c3[e]       # * r   -> G00
    alpha = c1[o] * c1[e]
    beta2 = c3[o] * c2[e]      # * r   -> G11
    alpha2 = c4[o] * c4[e]
    cdt = np.float16 if FP16 else np.float32
    rows = np.stack([g1, g2, h1, h2, beta, alpha, beta2, alpha2], axis=0).astype(cdt)
    out = np.zeros((NB, 8, NPAIR), cdt)
    for b in range(NB):
        out[b] = rows[:, b * NPAIR : (b + 1) * NPAIR]
    return out.reshape(1, -1)


def _prep_in_maps(X0, a, c, delta_bits, n):
    consts = _host_constants(a, c, n)
    dpad = np.zeros((NPAD, BS, 2), np.int32)
    dpad[:n] = delta_bits
    in_maps = []
    for ci in range(N_CORES):
        sl = slice(ci * BPC, (ci + 1) * BPC)
        d = dpad[:, sl, :].reshape(NB, T, P, C, 2).transpose(2, 0, 1, 3, 4)
        d = np.ascontiguousarray(d).reshape(P, NB * T * C * 2)
        x = np.ascontiguousarray(X0[sl].reshape(P, 2 * C))
        in_maps.append({"delta": d, "xin": x, "consts": consts})
    return in_maps


def _gather(results):
    out = np.empty((BS, 2), np.float32)
    for ci in range(N_CORES):
        y = results[ci]["yout"]
        sl = slice(ci * BPC, (ci + 1) * BPC)
        out[sl, 0] = y[:, 0:C].reshape(BPC)
        out[sl, 1] = y[:, C : 2 * C].reshape(BPC)
    return out


def kernel(X0, a, c, delta_bits, num_itr, **run_kwargs):
    X0 = np.ascontiguousarray(np.asarray(X0, np.float32))
    a = np.asarray(a, np.float32)
    c = np.asarray(c, np.float32)
    delta_bits = np.ascontiguousarray(np.asarray(delta_bits, np.int32))
    n = int(num_itr)
    assert X0.shape == (BS, 2) and delta_bits.shape == (n, BS, 2) and n == NIT

    if "nc" not in _CACHED:
        _CACHED["nc"] = _build_nc()
    nc = _CACHED["nc"]

    in_maps = _prep_in_maps(X0, a, c, delta_bits, n)
    res = run_bass_kernel_spmd(nc, in_maps, core_ids=list(range(N_CORES)), **run_kwargs)
    out = _gather(res.results)
    if run_kwargs:
        return out, res
    return out


if __name__ == "__main__":
    rng = np.random.default_rng(0)
    X0 = rng.random((BS, 2), dtype=np.float32)
    a = np.full((NIT,), 0.01, np.float32)
    c = np.full((NIT,), 0.01, np.float32)
    db = rng.integers(0, 2, size=(NIT, BS, 2), dtype=np.int32)
    out = kernel(X0=X0, a=a, c=c, delta_bits=db, num_itr=NIT)
    print("kernel ran, out:", out.shape, out.dtype, float(np.abs(out).max()))

